# revision 11
# baseline (speedup 1.0000x reference)
"""Behler-Parrinello NN energy kernel for 8 Trainium2 NeuronCores.

Strategy
--------
Data-parallel over atoms (125k H + 125k O per core). Host-side (numpy):
  * assigns every molecule (per core, per element) to one of 128 SBUF
    partitions with a count-balanced snake schedule,
  * ships features as fp16 in feature-major layout (fp8 shipping is a
    dead end on TRN2: e3m4 streams at 3 cyc/row through the PE and
    e4m3 fails the 2e-2 precision gate even with fp16 weights),
  * lays atoms out so the device MLP emits energies directly into the
    [128 x 992] per-element energy grid consumed by the gpsimd scatter.

Device-side per core (Bass/Tile), per 2048-atom block (all psum tiles
double-buffered or paired so the PE never idles long enough to drop
out of its ramped p-state clock):
  * L1: 4 col-packed fp16 matmuls -> psum [128, 1024] x2 bufs
    (2 atoms/col), tanh with fused b1 bias -> h1 fp16,
  * L2 (per block pair): 4 col-packed matmuls with block-diag W2 ->
    psum [128, 1024] (4 atoms/col), tanh + b2 -> h2 fp16,
  * L3: 8 tiny matmuls per pair, lhsT=h2-slice [128,128], rhs=4-col
    block-diag W3 -> psum energies [128, 4] per slice accumulating in
    a [128, 512] bank pair; DVE copies drop 256-col chunks into the
    fp16 energy grid every 8 pairs (replaces a DVE segmented-reduce
    that cost 85us), and the matching 248-col scatter batch fires
    immediately so the tail after the last block is short,
  * gpsimd local_scatter batches (640 bins per element half) + DVE adds
    accumulate molecular partial sums; host merges bins -> molecules,
    adds count*b3, sums the 8 cores.
"""

import sys

if "/opt/trn_rl_repo" not in sys.path:
    sys.path.insert(0, "/opt/trn_rl_repo")

import ml_dtypes
import numpy as np

# ---------------------------------------------------------------- constants
N_CORES = 8
N_MOL = 100_000
N_FEAT = 128
N_ATOMS = 1_000_000           # per element, global
APC = N_ATOMS // N_CORES      # atoms per core per element (125000)

T_COLS = 992                  # energy-grid columns per partition per element
SLOTS = 128 * T_COLS          # atom slots per core per element (126976)
BLOCKS = 62                   # 2048-atom blocks per element
BLK_X = 2048                  # x columns per block
NB = 4                        # scatter batches per element
BW = T_COLS // NB             # columns per batch (248)
N_BINS = 1280                 # acc bins per partition (H: [0,640), O: [640,1280))
BIN_HALF = 640

# fp8 feature shipping is dead on TRN2: e3m4 streams at 3 cyc/row on the PE
# (measured 635ns per 512-col matmul) and e4m3 fails the precision gate
# (1.94e-2 vs 2e-2 even with fp16 weights). Features ship as fp16.
X_SCALE = 1.0                 # host scale on features
W1_SCALE = 1.0                # host scale on W1
X_DT = np.float16

_CACHE = {}


# ================================================================ device IR
def _build_nc():
    import concourse.bacc as bacc
    import concourse.mybir as mybir
    from concourse.tile import TileContext

    dt = mybir.dt
    f16, f32, i16 = dt.float16, dt.float32, dt.int16
    Tanh = mybir.ActivationFunctionType.Tanh

    nc = bacc.Bacc("TRN2", target_bir_lowering=False, debug=False)

    xt = {
        e: nc.dram_tensor(f"xt_{e}", [128, SLOTS], f16, kind="ExternalInput")
        for e in ("h", "o")
    }
    w1 = {
        e: nc.dram_tensor(f"w1_{e}", [128, 64], f16, kind="ExternalInput")
        for e in ("h", "o")
    }
    wf = {
        e: nc.dram_tensor(f"wf_{e}", [128, 68], f16, kind="ExternalInput")
        for e in ("h", "o")
    }
    bpk = {
        e: nc.dram_tensor(f"bpk_{e}", [128, 2], f32, kind="ExternalInput")
        for e in ("h", "o")
    }
    q_idx = nc.dram_tensor("q_idx", [128, 2 * T_COLS], i16, kind="ExternalInput")
    out_acc = nc.dram_tensor("out_acc", [128, N_BINS], f32, kind="ExternalOutput")

    with TileContext(nc) as tc:
        with (
            tc.tile_pool(name="wpool", bufs=1) as wpool,
            tc.tile_pool(name="xpool", bufs=6) as xpool,
            tc.tile_pool(name="h1pool", bufs=4) as h1pool,
            tc.tile_pool(name="h2pool", bufs=2) as h2pool,
            tc.tile_pool(name="epool", bufs=1) as epool,
            tc.tile_pool(name="spool", bufs=2) as spool,
            tc.tile_pool(name="ps1", bufs=2, space="PSUM") as ps1,
            tc.tile_pool(name="ps2", bufs=1, space="PSUM") as ps2,
            tc.tile_pool(name="ps3", bufs=2, space="PSUM") as ps3,
        ):
            # --- persistent tiles
            E = epool.tile([128, 2 * T_COLS], f16, tag="E")
            Q = epool.tile([128, 2 * T_COLS], i16, tag="Q")
            acc = epool.tile([128, N_BINS], f32, tag="acc")
            nc.vector.memset(acc[:], 0.0)

            warm = epool.tile([128, 1], f32, tag="warm")
            nc.scalar.activation(warm[:], acc[:, 0:1], Tanh)

            wt = {}
            wtiles = {}
            for e in ("h", "o"):
                w1t = wpool.tile([128, 64], f16, tag=f"w1{e}", name=f"w1{e}")
                wft = wpool.tile([128, 68], f16, tag=f"wf{e}", name=f"wf{e}")
                bt = wpool.tile([128, 2], f32, tag=f"b{e}", name=f"b{e}")
                wtiles[e] = (w1t, wft, bt)
                wt[e] = {
                    "w1": w1t[:],
                    "w2s": wft[:, 0:64],
                    "b4": wft[:, 64:68],
                    "b1s": bt[:, 0:1],
                    "b2s": bt[:, 1:2],
                }

            def load_weights(e):
                w1t, wft, bt = wtiles[e]
                nc.sync.dma_start(w1t[:], w1[e][:])
                nc.sync.dma_start(wft[:], wf[e][:])
                nc.sync.dma_start(bt[:], bpk[e][:])

            load_weights("h")
            load_weights("o")
            nc.sync.dma_start(Q[:], q_idx[:])

            for ei, e in enumerate(("h", "o")):
                W = wt[e]
                p3 = None
                h1pair = [None, None]
                for b in range(BLOCKS):
                    xtile = xpool.tile([128, BLK_X], f16, tag="xt", name=f"xt{e}{b}")
                    nc.sync.dma_start(
                        xtile[:], xt[e][:, b * BLK_X : (b + 1) * BLK_X]
                    )
                    # L1: 4 col-packed matmuls; x col 1024*g2+512*blk+c ->
                    # p1[64*blk + feat, 512*g2 + c]
                    p1 = ps1.tile([128, 1024], f32, tag="p1", name=f"p1_{e}{b}")
                    for g2 in range(2):
                        for blk in range(2):
                            o = 1024 * g2 + 512 * blk
                            nc.tensor.matmul(
                                p1[64 * blk : 64 * blk + 64,
                                   512 * g2 : 512 * (g2 + 1)],
                                W["w1"],
                                xtile[:, o : o + 512],
                                tile_position=(0, 64 * blk),
                            )
                    h1 = h1pool.tile([128, 1024], f16, tag="h1", name=f"h1_{e}{b}")
                    nc.scalar.activation(
                        h1[:], p1[:], Tanh, bias=W["b1s"],
                        scale=1.0 / (X_SCALE * W1_SCALE),
                    )
                    h1pair[b % 2] = h1
                    if b % 2 == 0:
                        continue
                    # L2 over the block pair (b-1, b): 4 col-packed matmuls;
                    # h1(b-1+pb) col 512*sb+c -> p2[64*sb + 32*half + k, 512*pb + c]
                    p2 = ps2.tile([128, 1024], f32, tag="p2", name=f"p2_{e}{b}")
                    for pb in range(2):
                        for sb in range(2):
                            nc.tensor.matmul(
                                p2[64 * sb : 64 * sb + 64,
                                   512 * pb : 512 * (pb + 1)],
                                W["w2s"],
                                h1pair[pb][:, 512 * sb : 512 * sb + 512],
                                tile_position=(0, 64 * sb),
                            )
                    h2 = h2pool.tile([128, 1024], f16, tag="h2", name=f"h2_{e}{b}")
                    nc.scalar.activation(h2[:], p2[:], Tanh, bias=W["b2s"])
                    # L3: energies; h2 col 128*s3+m, row 64*sb+32*half+k ->
                    # p3[m, 32*(m_pair-pbase) + 4*s3 + 2*sb+half]
                    mp = b // 2
                    if mp % 16 == 0:
                        p3 = ps3.tile([128, 512], f32, tag="p3", name=f"p3_{e}{b}")
                        pbase = mp
                    off = 32 * (mp - pbase)
                    for s3 in range(8):
                        nc.tensor.matmul(
                            p3[:, off + 4 * s3 : off + 4 * s3 + 4],
                            h2[:, 128 * s3 : 128 * s3 + 128],
                            W["b4"],
                            tile_position=(0, 0),
                        )
                    if b == 31:
                        with nc.allow_low_precision("fp16 energy grid"):
                            nc.vector.tensor_scalar_add(
                                E[:, ei * T_COLS : ei * T_COLS + 512],
                                p3[:], 0.0,
                            )
                    elif b == 61:
                        with nc.allow_low_precision("fp16 energy grid"):
                            nc.vector.tensor_scalar_add(
                                E[:, ei * T_COLS + 512 : ei * T_COLS + T_COLS],
                                p3[:, 0 : T_COLS - 512], 0.0,
                            )
                        # scatter this element's energies into 640-bin halves
                        for bi in range(NB):
                            r0 = ei * T_COLS + bi * BW
                            S = spool.tile(
                                [128, BIN_HALF], f16, tag="S", name=f"S{e}{bi}"
                            )
                            nc.gpsimd.local_scatter(
                                S[:],
                                E[:, r0 : r0 + BW],
                                Q[:, r0 : r0 + BW],
                                channels=128,
                                num_elems=BIN_HALF,
                                num_idxs=BW,
                            )
                            nc.vector.tensor_tensor(
                                acc[:, ei * BIN_HALF : (ei + 1) * BIN_HALF],
                                acc[:, ei * BIN_HALF : (ei + 1) * BIN_HALF],
                                S[:],
                                op=mybir.AluOpType.add,
                            )

            nc.sync.dma_start(out_acc[:], acc[:])

    nc.compile()
    return nc


# ================================================================ host plan
def _device_slot_to_xcol():
    """F: energy-grid slot (partition p, col e) -> x column, per element.

    Derived from the device dataflow (2048-atom blocks, paired for L2):
      e = 32*m + 4*s3 + g,  g = 2*sb + half
      P2 = 128*s3 + p ; pb = P2//512 ; c = P2%512
      xcol = 2048*(2*m+pb) + 1024*sb + 512*half + c
    """
    p = np.arange(128)[:, None]
    e = np.arange(T_COLS)[None, :]
    m = e // 32
    w = e % 32
    s3 = w // 4
    g = w % 4
    sb = g // 2
    half = g % 2
    P2 = 128 * s3 + p
    pb = P2 // 512
    c = P2 % 512
    return (2048 * (2 * m + pb) + 1024 * sb + 512 * half + c).astype(np.int64)


_F_MAP = _device_slot_to_xcol()


def _plan_element(m):
    """Plan one (core, element): molecule->partition, atom->slot, bins.

    m: int32 [n] molecule index per atom (core's shard).
    Returns (perm_x, q, bin_mol, bin_p, bin_id) where
      perm_x int64 [SLOTS] source atom per x column (pads -> 0),
      q      int16 [128, T_COLS] bin per slot (-1 for pads), bins in [0, 640),
      bin_mol/bin_p/bin_id: molecule ids and (partition, bin) locations
      for the host-side merge.
    """
    n = m.shape[0]
    cnt = np.bincount(m, minlength=N_MOL)
    present = np.flatnonzero(cnt)
    # snake assignment of present molecules (count-desc) to partitions
    order = present[np.argsort(-cnt[present], kind="stable")]
    r = np.arange(order.size)
    pat = r % 256
    p_of_rank = np.where(pat < 128, pat, 255 - pat)
    p_assign = np.full(N_MOL, -1, np.int32)
    p_assign[order] = p_of_rank
    # primary bin = rank of molecule within its partition (by snake order)
    prim = np.full(N_MOL, -1, np.int32)
    o2 = np.argsort(p_of_rank, kind="stable")
    pp = p_of_rank[o2]
    starts = np.searchsorted(pp, np.arange(128))
    within = np.arange(order.size) - starts[pp]
    prim[order[o2]] = within
    n_prim = np.bincount(pp, minlength=128)

    # per-atom occurrence rank k within its molecule
    a_sort = np.argsort(m, kind="stable")
    ms = m[a_sort]
    gstart = np.r_[0, np.flatnonzero(np.diff(ms)) + 1]
    glen = np.diff(np.r_[gstart, n])
    k = np.arange(n) - np.repeat(gstart, glen)
    level = k // NB

    # spill bins for occurrences beyond NB per molecule
    bins_sorted = prim[ms].copy()
    sp_first = (level >= 1) & (k % NB == 0)
    if sp_first.any():
        sp_pos = np.flatnonzero(sp_first)
        sp_p = p_assign[ms[sp_pos]]
        so = np.argsort(sp_p, kind="stable")
        sp_sorted_p = sp_p[so]
        sp_starts = np.searchsorted(sp_sorted_p, np.arange(128))
        sp_within = np.arange(sp_pos.size) - sp_starts[sp_sorted_p]
        sp_bin = np.empty(sp_pos.size, np.int32)
        sp_bin[so] = n_prim[sp_sorted_p] + sp_within
        gid = np.cumsum(sp_first) - 1
        lvl_mask = level >= 1
        bins_sorted[lvl_mask] = sp_bin[gid[lvl_mask]]
        sp_mol = ms[sp_pos]
        sp_part = p_assign[sp_mol]
    else:
        sp_bin = np.empty(0, np.int32)
        sp_mol = np.empty(0, np.int32)
        sp_part = np.empty(0, np.int32)

    p_atom = p_assign[ms]
    # position within partition: sort by (partition, bin, k)
    o3 = np.lexsort((k, bins_sorted, p_atom))
    p3 = p_atom[o3]
    pstarts = np.searchsorted(p3, np.arange(128))
    pos = np.arange(n) - pstarts[p3]
    load = np.bincount(p3, minlength=128)
    if load.max() > T_COLS:
        raise RuntimeError(f"partition overload {load.max()} > {T_COLS}")
    nb_used = int(n_prim.max() + (np.bincount(sp_part, minlength=128).max()
                                  if sp_part.size else 0))
    if nb_used > BIN_HALF:
        raise RuntimeError(f"bins overload {nb_used} > {BIN_HALF}")

    batch = pos % NB
    col = batch * BW + pos // NB
    atom_ids = a_sort[o3]

    perm_x = np.zeros(SLOTS, np.int64)
    q = np.full((128, T_COLS), -1, np.int16)
    perm_x[_F_MAP[p3, col]] = atom_ids
    q[p3, col] = bins_sorted[o3]

    bin_mol = np.concatenate([order, sp_mol])
    bin_p = np.concatenate([p_of_rank, sp_part])
    bin_id = np.concatenate([prim[order], sp_bin])
    return perm_x, q, bin_mol, bin_p, bin_id


def _prep_weights(W1, b1, W2, b2, W3):
    w1q = np.ascontiguousarray(
        (np.asarray(W1, np.float32) * W1_SCALE).astype(X_DT)
    )                                                               # [128, 64]
    w2s = np.zeros((128, 64), np.float32)                           # block-diag
    w2s[0:64, 0:32] = W2
    w2s[64:128, 32:64] = W2
    b4 = np.zeros((128, 4), np.float32)                             # 4-block W3
    w3c = np.asarray(W3, np.float32)[:, 0]
    for g in range(4):
        b4[32 * g : 32 * g + 32, g] = w3c
    wf = np.ascontiguousarray(np.hstack([w2s, b4]), np.float16)     # [128, 68]
    b1c = np.asarray(b1, np.float32).reshape(-1, 1)
    b2c = np.asarray(b2, np.float32).reshape(-1, 1)
    b1s = np.vstack([b1c, b1c]).astype(np.float32)                  # [128, 1]
    b2s = np.vstack([b2c] * 4).astype(np.float32)                   # [128, 1]
    bpk = np.ascontiguousarray(np.hstack([b1s, b2s]), np.float32)
    return w1q, wf, bpk


# ================================================================ entry
def _prepare(
    feats_H, feats_O, mol_idx_H, mol_idx_O,
    W1_H, b1_H, W2_H, b2_H, W3_H,
    W1_O, b1_O, W2_O, b2_O, W3_O,
):
    feats = {"h": np.asarray(feats_H), "o": np.asarray(feats_O)}
    mols = {
        "h": np.asarray(mol_idx_H, np.int32),
        "o": np.asarray(mol_idx_O, np.int32),
    }
    wts = {
        "h": _prep_weights(W1_H, b1_H, W2_H, b2_H, W3_H),
        "o": _prep_weights(W1_O, b1_O, W2_O, b2_O, W3_O),
    }

    in_maps = []
    merge = []            # per core: [(bin_mol, bin_p, bin_col)] per element
    for c in range(N_CORES):
        im = {}
        mg = []
        q_full = np.empty((128, 2 * T_COLS), np.int16)
        for ei, e in enumerate(("h", "o")):
            sl = slice(c * APC, (c + 1) * APC)
            perm_x, q, bm, bp, bid = _plan_element(mols[e][sl])
            xs = feats[e][sl]
            xtp = np.ascontiguousarray(
                (np.asarray(xs, np.float32)[perm_x] * X_SCALE).astype(X_DT).T
            )                                            # [128, SLOTS]
            im[f"xt_{e}"] = xtp
            q_full[:, ei * T_COLS : (ei + 1) * T_COLS] = q
            mg.append((bm, bp, bid + ei * BIN_HALF))
            im[f"w1_{e}"], im[f"wf_{e}"], im[f"bpk_{e}"] = wts[e]
        im["q_idx"] = q_full
        in_maps.append(im)
        merge.append(mg)
    return in_maps, merge


def kernel(
    feats_H, feats_O, mol_idx_H, mol_idx_O, n_molecules,
    W1_H, b1_H, W2_H, b2_H, W3_H, b3_H,
    W1_O, b1_O, W2_O, b2_O, W3_O, b3_O,
):
    from concourse import bass_utils

    in_maps, merge = _prepare(
        feats_H, feats_O, mol_idx_H, mol_idx_O,
        W1_H, b1_H, W2_H, b2_H, W3_H,
        W1_O, b1_O, W2_O, b2_O, W3_O,
    )
    if "nc" not in _CACHE:
        _CACHE["nc"] = _build_nc()
    nc = _CACHE["nc"]

    _CACHE["in_maps"] = in_maps
    res = bass_utils.run_bass_kernel_spmd(
        nc, in_maps, core_ids=list(range(N_CORES))
    )

    mols = {
        "h": np.asarray(mol_idx_H, np.int32),
        "o": np.asarray(mol_idx_O, np.int32),
    }
    out = np.zeros(N_MOL, np.float64)
    for c in range(N_CORES):
        acc = res.results[c]["out_acc"]
        for bm, bp, bid in merge[c]:
            out += np.bincount(
                bm, weights=acc[bp, bid].astype(np.float64), minlength=N_MOL
            )
    cnt_h = np.bincount(mols["h"], minlength=N_MOL)
    cnt_o = np.bincount(mols["o"], minlength=N_MOL)
    out += cnt_h * float(np.asarray(b3_H).reshape(()))
    out += cnt_o * float(np.asarray(b3_O).reshape(()))
    return out.astype(np.float32)


# revision 12
# speedup vs baseline: 1.1904x; 1.1904x over previous
"""Behler-Parrinello NN energy kernel for 8 Trainium2 NeuronCores.

Strategy
--------
Data-parallel over atoms (125k H + 125k O per core). Host-side (numpy):
  * assigns every molecule (per core, per element) to one of 128 SBUF
    partitions with a count-balanced snake schedule,
  * ships features as fp16 in feature-major layout (fp8 shipping is a
    dead end on TRN2: e3m4 streams at 3 cyc/row through the PE and
    e4m3 fails the 2e-2 precision gate even with fp16 weights),
  * lays atoms out so the device MLP emits energies directly into the
    [128 x 992] per-element energy grid consumed by the gpsimd scatter.

Device-side per core (Bass/Tile), per 2048-atom block (all psum tiles
double-buffered or paired so the PE never idles long enough to drop
out of its ramped p-state clock):
  * L1: 4 col-packed fp16 matmuls -> psum [128, 1024] x2 bufs
    (2 atoms/col), tanh with fused b1 bias -> h1 fp16,
  * L2 (per block pair): 4 col-packed matmuls with block-diag W2 ->
    psum [128, 1024] (4 atoms/col), tanh + b2 -> h2 fp16,
  * L3: 8 tiny matmuls per pair, lhsT=h2-slice [128,128], rhs=4-col
    block-diag W3 -> psum energies [128, 4] per slice accumulating in
    a [128, 512] bank pair; DVE copies drop 256-col chunks into the
    fp16 energy grid every 8 pairs (replaces a DVE segmented-reduce
    that cost 85us), and the matching 248-col scatter batch fires
    immediately so the tail after the last block is short,
  * gpsimd local_scatter batches (640 bins per element half) + DVE adds
    accumulate molecular partial sums; host merges bins -> molecules,
    adds count*b3, sums the 8 cores.
"""

import sys

if "/opt/trn_rl_repo" not in sys.path:
    sys.path.insert(0, "/opt/trn_rl_repo")

import ml_dtypes
import numpy as np

# ---------------------------------------------------------------- constants
N_CORES = 8
N_MOL = 100_000
N_FEAT = 128
N_ATOMS = 1_000_000           # per element, global
APC = N_ATOMS // N_CORES      # atoms per core per element (125000)

T_COLS = 992                  # energy-grid columns per partition per element
SLOTS = 128 * T_COLS          # atom slots per core per element (126976)
BLOCKS = 62                   # 2048-atom blocks per element
BLK_X = 2048                  # x columns per block
NB = 4                        # scatter batches per element
BW = T_COLS // NB             # columns per batch (248)
N_BINS = 1280                 # acc bins per partition (H: [0,640), O: [640,1280))
BIN_HALF = 640

# fp8 feature shipping is dead on TRN2: e3m4 streams at 3 cyc/row on the PE
# (measured 635ns per 512-col matmul) and e4m3 fails the precision gate
# (1.94e-2 vs 2e-2 even with fp16 weights). Features ship as fp16.
X_SCALE = 1.0                 # host scale on features
W1_SCALE = 1.0                # host scale on W1
X_DT = np.float16

_CACHE = {}


# ================================================================ device IR
def _build_nc():
    import concourse.bacc as bacc
    import concourse.mybir as mybir
    from concourse.tile import TileContext

    dt = mybir.dt
    f16, f32, i16 = dt.float16, dt.float32, dt.int16
    Tanh = mybir.ActivationFunctionType.Tanh

    nc = bacc.Bacc("TRN2", target_bir_lowering=False, debug=False)

    xt = {
        e: nc.dram_tensor(f"xt_{e}", [128, SLOTS], f16, kind="ExternalInput")
        for e in ("h", "o")
    }
    w1 = {
        e: nc.dram_tensor(f"w1_{e}", [128, 64], f16, kind="ExternalInput")
        for e in ("h", "o")
    }
    wf = {
        e: nc.dram_tensor(f"wf_{e}", [128, 68], f16, kind="ExternalInput")
        for e in ("h", "o")
    }
    bpk = {
        e: nc.dram_tensor(f"bpk_{e}", [128, 2], f32, kind="ExternalInput")
        for e in ("h", "o")
    }
    q_idx = nc.dram_tensor("q_idx", [128, 2 * T_COLS], i16, kind="ExternalInput")
    out_acc = nc.dram_tensor("out_acc", [128, N_BINS], f32, kind="ExternalOutput")

    with TileContext(nc) as tc:
        with (
            tc.tile_pool(name="wpool", bufs=1) as wpool,
            tc.tile_pool(name="xpool", bufs=6) as xpool,
            tc.tile_pool(name="h1pool", bufs=4) as h1pool,
            tc.tile_pool(name="h2pool", bufs=2) as h2pool,
            tc.tile_pool(name="epool", bufs=1) as epool,
            tc.tile_pool(name="spool", bufs=2) as spool,
            tc.tile_pool(name="ps1", bufs=2, space="PSUM") as ps1,
            tc.tile_pool(name="ps2", bufs=1, space="PSUM") as ps2,
            tc.tile_pool(name="ps3", bufs=2, space="PSUM") as ps3,
        ):
            # --- persistent tiles
            E = epool.tile([128, 2 * T_COLS], f16, tag="E")
            Q = epool.tile([128, 2 * T_COLS], i16, tag="Q")
            acc = epool.tile([128, N_BINS], f32, tag="acc")
            nc.vector.memset(acc[:], 0.0)

            warm = epool.tile([128, 1], f32, tag="warm")
            nc.scalar.activation(warm[:], acc[:, 0:1], Tanh)

            wt = {}
            wtiles = {}
            for e in ("h", "o"):
                w1t = wpool.tile([128, 64], f16, tag=f"w1{e}", name=f"w1{e}")
                wft = wpool.tile([128, 68], f16, tag=f"wf{e}", name=f"wf{e}")
                bt = wpool.tile([128, 2], f32, tag=f"b{e}", name=f"b{e}")
                wtiles[e] = (w1t, wft, bt)
                wt[e] = {
                    "w1": w1t[:],
                    "w2s": wft[:, 0:64],
                    "b4": wft[:, 64:68],
                    "b1s": bt[:, 0:1],
                    "b2s": bt[:, 1:2],
                }

            def load_weights(e):
                w1t, wft, bt = wtiles[e]
                nc.sync.dma_start(w1t[:], w1[e][:])
                nc.sync.dma_start(wft[:], wf[e][:])
                nc.sync.dma_start(bt[:], bpk[e][:])

            nc.sync.dma_start(Q[:], q_idx[:])
            load_weights("h")
            load_weights("o")

            for ei, e in enumerate(("h", "o")):
                W = wt[e]
                p3 = None
                h1pair = [None, None]
                for b in range(BLOCKS):
                    xtile = xpool.tile([128, BLK_X], f16, tag="xt", name=f"xt{e}{b}")
                    nc.sync.dma_start(
                        xtile[:], xt[e][:, b * BLK_X : (b + 1) * BLK_X]
                    )
                    # L1: 4 col-packed matmuls; x col 1024*g2+512*blk+c ->
                    # p1[64*blk + feat, 512*g2 + c]
                    p1 = ps1.tile([128, 1024], f32, tag="p1", name=f"p1_{e}{b}")
                    for g2 in range(2):
                        for blk in range(2):
                            o = 1024 * g2 + 512 * blk
                            nc.tensor.matmul(
                                p1[64 * blk : 64 * blk + 64,
                                   512 * g2 : 512 * (g2 + 1)],
                                W["w1"],
                                xtile[:, o : o + 512],
                                tile_position=(0, 64 * blk),
                            )
                    h1 = h1pool.tile([128, 1024], f16, tag="h1", name=f"h1_{e}{b}")
                    nc.scalar.activation(
                        h1[:], p1[:], Tanh, bias=W["b1s"],
                        scale=1.0 / (X_SCALE * W1_SCALE),
                    )
                    h1pair[b % 2] = h1
                    if b % 2 == 0:
                        continue
                    # L2 over the block pair (b-1, b): 4 col-packed matmuls;
                    # h1(b-1+pb) col 512*sb+c -> p2[64*sb + 32*half + k, 512*pb + c]
                    p2 = ps2.tile([128, 1024], f32, tag="p2", name=f"p2_{e}{b}")
                    for pb in range(2):
                        for sb in range(2):
                            nc.tensor.matmul(
                                p2[64 * sb : 64 * sb + 64,
                                   512 * pb : 512 * (pb + 1)],
                                W["w2s"],
                                h1pair[pb][:, 512 * sb : 512 * sb + 512],
                                tile_position=(0, 64 * sb),
                            )
                    h2 = h2pool.tile([128, 1024], f16, tag="h2", name=f"h2_{e}{b}")
                    nc.scalar.activation(h2[:], p2[:], Tanh, bias=W["b2s"])
                    # L3: energies; h2 col 128*s3+m, row 64*sb+32*half+k ->
                    # p3[m, 32*(m_pair-pbase) + 4*s3 + 2*sb+half]
                    mp = b // 2
                    if mp % 16 == 0:
                        p3 = ps3.tile([128, 512], f32, tag="p3", name=f"p3_{e}{b}")
                        pbase = mp
                    off = 32 * (mp - pbase)
                    for s3 in range(8):
                        nc.tensor.matmul(
                            p3[:, off + 4 * s3 : off + 4 * s3 + 4],
                            h2[:, 128 * s3 : 128 * s3 + 128],
                            W["b4"],
                            tile_position=(0, 0),
                        )
                    if b == 31:
                        with nc.allow_low_precision("fp16 energy grid"):
                            nc.vector.tensor_scalar_add(
                                E[:, ei * T_COLS : ei * T_COLS + 512],
                                p3[:], 0.0,
                            )
                    elif b == 61:
                        with nc.allow_low_precision("fp16 energy grid"):
                            nc.vector.tensor_scalar_add(
                                E[:, ei * T_COLS + 512 : ei * T_COLS + T_COLS],
                                p3[:, 0 : T_COLS - 512], 0.0,
                            )
                        # scatter this element's energies into 640-bin halves
                        for bi in range(NB):
                            r0 = ei * T_COLS + bi * BW
                            S = spool.tile(
                                [128, BIN_HALF], f16, tag="S", name=f"S{e}{bi}"
                            )
                            nc.gpsimd.local_scatter(
                                S[:],
                                E[:, r0 : r0 + BW],
                                Q[:, r0 : r0 + BW],
                                channels=128,
                                num_elems=BIN_HALF,
                                num_idxs=BW,
                            )
                            nc.vector.tensor_tensor(
                                acc[:, ei * BIN_HALF : (ei + 1) * BIN_HALF],
                                acc[:, ei * BIN_HALF : (ei + 1) * BIN_HALF],
                                S[:],
                                op=mybir.AluOpType.add,
                            )

            nc.sync.dma_start(out_acc[:], acc[:])

    nc.compile()
    return nc


# ================================================================ host plan
def _device_slot_to_xcol():
    """F: energy-grid slot (partition p, col e) -> x column, per element.

    Derived from the device dataflow (2048-atom blocks, paired for L2):
      e = 32*m + 4*s3 + g,  g = 2*sb + half
      P2 = 128*s3 + p ; pb = P2//512 ; c = P2%512
      xcol = 2048*(2*m+pb) + 1024*sb + 512*half + c
    """
    p = np.arange(128)[:, None]
    e = np.arange(T_COLS)[None, :]
    m = e // 32
    w = e % 32
    s3 = w // 4
    g = w % 4
    sb = g // 2
    half = g % 2
    P2 = 128 * s3 + p
    pb = P2 // 512
    c = P2 % 512
    return (2048 * (2 * m + pb) + 1024 * sb + 512 * half + c).astype(np.int64)


_F_MAP = _device_slot_to_xcol()


def _plan_element(m):
    """Plan one (core, element): molecule->partition, atom->slot, bins.

    m: int32 [n] molecule index per atom (core's shard).
    Returns (perm_x, q, bin_mol, bin_p, bin_id) where
      perm_x int64 [SLOTS] source atom per x column (pads -> 0),
      q      int16 [128, T_COLS] bin per slot (-1 for pads), bins in [0, 640),
      bin_mol/bin_p/bin_id: molecule ids and (partition, bin) locations
      for the host-side merge.
    """
    n = m.shape[0]
    cnt = np.bincount(m, minlength=N_MOL)
    present = np.flatnonzero(cnt)
    # snake assignment of present molecules (count-desc) to partitions
    order = present[np.argsort(-cnt[present], kind="stable")]
    r = np.arange(order.size)
    pat = r % 256
    p_of_rank = np.where(pat < 128, pat, 255 - pat)
    p_assign = np.full(N_MOL, -1, np.int32)
    p_assign[order] = p_of_rank
    # primary bin = rank of molecule within its partition (by snake order)
    prim = np.full(N_MOL, -1, np.int32)
    o2 = np.argsort(p_of_rank, kind="stable")
    pp = p_of_rank[o2]
    starts = np.searchsorted(pp, np.arange(128))
    within = np.arange(order.size) - starts[pp]
    prim[order[o2]] = within
    n_prim = np.bincount(pp, minlength=128)

    # per-atom occurrence rank k within its molecule
    a_sort = np.argsort(m, kind="stable")
    ms = m[a_sort]
    gstart = np.r_[0, np.flatnonzero(np.diff(ms)) + 1]
    glen = np.diff(np.r_[gstart, n])
    k = np.arange(n) - np.repeat(gstart, glen)
    level = k // NB

    # spill bins for occurrences beyond NB per molecule
    bins_sorted = prim[ms].copy()
    sp_first = (level >= 1) & (k % NB == 0)
    if sp_first.any():
        sp_pos = np.flatnonzero(sp_first)
        sp_p = p_assign[ms[sp_pos]]
        so = np.argsort(sp_p, kind="stable")
        sp_sorted_p = sp_p[so]
        sp_starts = np.searchsorted(sp_sorted_p, np.arange(128))
        sp_within = np.arange(sp_pos.size) - sp_starts[sp_sorted_p]
        sp_bin = np.empty(sp_pos.size, np.int32)
        sp_bin[so] = n_prim[sp_sorted_p] + sp_within
        gid = np.cumsum(sp_first) - 1
        lvl_mask = level >= 1
        bins_sorted[lvl_mask] = sp_bin[gid[lvl_mask]]
        sp_mol = ms[sp_pos]
        sp_part = p_assign[sp_mol]
    else:
        sp_bin = np.empty(0, np.int32)
        sp_mol = np.empty(0, np.int32)
        sp_part = np.empty(0, np.int32)

    p_atom = p_assign[ms]
    # position within partition: sort by (partition, bin, k)
    o3 = np.lexsort((k, bins_sorted, p_atom))
    p3 = p_atom[o3]
    pstarts = np.searchsorted(p3, np.arange(128))
    pos = np.arange(n) - pstarts[p3]
    load = np.bincount(p3, minlength=128)
    if load.max() > T_COLS:
        raise RuntimeError(f"partition overload {load.max()} > {T_COLS}")
    nb_used = int(n_prim.max() + (np.bincount(sp_part, minlength=128).max()
                                  if sp_part.size else 0))
    if nb_used > BIN_HALF:
        raise RuntimeError(f"bins overload {nb_used} > {BIN_HALF}")

    batch = pos % NB
    col = batch * BW + pos // NB
    atom_ids = a_sort[o3]

    perm_x = np.zeros(SLOTS, np.int64)
    q = np.full((128, T_COLS), -1, np.int16)
    perm_x[_F_MAP[p3, col]] = atom_ids
    q[p3, col] = bins_sorted[o3]

    bin_mol = np.concatenate([order, sp_mol])
    bin_p = np.concatenate([p_of_rank, sp_part])
    bin_id = np.concatenate([prim[order], sp_bin])
    return perm_x, q, bin_mol, bin_p, bin_id


def _prep_weights(W1, b1, W2, b2, W3):
    w1q = np.ascontiguousarray(
        (np.asarray(W1, np.float32) * W1_SCALE).astype(X_DT)
    )                                                               # [128, 64]
    w2s = np.zeros((128, 64), np.float32)                           # block-diag
    w2s[0:64, 0:32] = W2
    w2s[64:128, 32:64] = W2
    b4 = np.zeros((128, 4), np.float32)                             # 4-block W3
    w3c = np.asarray(W3, np.float32)[:, 0]
    for g in range(4):
        b4[32 * g : 32 * g + 32, g] = w3c
    wf = np.ascontiguousarray(np.hstack([w2s, b4]), np.float16)     # [128, 68]
    b1c = np.asarray(b1, np.float32).reshape(-1, 1)
    b2c = np.asarray(b2, np.float32).reshape(-1, 1)
    b1s = np.vstack([b1c, b1c]).astype(np.float32)                  # [128, 1]
    b2s = np.vstack([b2c] * 4).astype(np.float32)                   # [128, 1]
    bpk = np.ascontiguousarray(np.hstack([b1s, b2s]), np.float32)
    return w1q, wf, bpk


# ================================================================ entry
def _prepare(
    feats_H, feats_O, mol_idx_H, mol_idx_O,
    W1_H, b1_H, W2_H, b2_H, W3_H,
    W1_O, b1_O, W2_O, b2_O, W3_O,
):
    feats = {"h": np.asarray(feats_H), "o": np.asarray(feats_O)}
    mols = {
        "h": np.asarray(mol_idx_H, np.int32),
        "o": np.asarray(mol_idx_O, np.int32),
    }
    wts = {
        "h": _prep_weights(W1_H, b1_H, W2_H, b2_H, W3_H),
        "o": _prep_weights(W1_O, b1_O, W2_O, b2_O, W3_O),
    }

    in_maps = []
    merge = []            # per core: [(bin_mol, bin_p, bin_col)] per element
    for c in range(N_CORES):
        im = {}
        mg = []
        q_full = np.empty((128, 2 * T_COLS), np.int16)
        for ei, e in enumerate(("h", "o")):
            sl = slice(c * APC, (c + 1) * APC)
            perm_x, q, bm, bp, bid = _plan_element(mols[e][sl])
            xs = feats[e][sl]
            xtp = np.ascontiguousarray(
                (np.asarray(xs, np.float32)[perm_x] * X_SCALE).astype(X_DT).T
            )                                            # [128, SLOTS]
            im[f"xt_{e}"] = xtp
            q_full[:, ei * T_COLS : (ei + 1) * T_COLS] = q
            mg.append((bm, bp, bid + ei * BIN_HALF))
            im[f"w1_{e}"], im[f"wf_{e}"], im[f"bpk_{e}"] = wts[e]
        im["q_idx"] = q_full
        in_maps.append(im)
        merge.append(mg)
    return in_maps, merge


def kernel(
    feats_H, feats_O, mol_idx_H, mol_idx_O, n_molecules,
    W1_H, b1_H, W2_H, b2_H, W3_H, b3_H,
    W1_O, b1_O, W2_O, b2_O, W3_O, b3_O,
):
    from concourse import bass_utils

    in_maps, merge = _prepare(
        feats_H, feats_O, mol_idx_H, mol_idx_O,
        W1_H, b1_H, W2_H, b2_H, W3_H,
        W1_O, b1_O, W2_O, b2_O, W3_O,
    )
    if "nc" not in _CACHE:
        _CACHE["nc"] = _build_nc()
    nc = _CACHE["nc"]

    _CACHE["in_maps"] = in_maps
    res = bass_utils.run_bass_kernel_spmd(
        nc, in_maps, core_ids=list(range(N_CORES))
    )

    mols = {
        "h": np.asarray(mol_idx_H, np.int32),
        "o": np.asarray(mol_idx_O, np.int32),
    }
    out = np.zeros(N_MOL, np.float64)
    for c in range(N_CORES):
        acc = res.results[c]["out_acc"]
        for bm, bp, bid in merge[c]:
            out += np.bincount(
                bm, weights=acc[bp, bid].astype(np.float64), minlength=N_MOL
            )
    cnt_h = np.bincount(mols["h"], minlength=N_MOL)
    cnt_o = np.bincount(mols["o"], minlength=N_MOL)
    out += cnt_h * float(np.asarray(b3_H).reshape(()))
    out += cnt_o * float(np.asarray(b3_O).reshape(()))
    return out.astype(np.float32)
